# revision 7
# baseline (speedup 1.0000x reference)
"""BiLSTM encoder Trainium2 kernel (8-core SPMD, batch-data-parallel).

Strategy (v2)
-------------
- Shard batch B=128 across 8 cores (16 per core); replicate weights.
- Host folds the input projection into the gate projection and
  pre-transposes x to feature-major bf16:
      pre = Wc @ x^T + bc,   Wc = (Wih @ W_proj) (permuted/scaled)
  so the device GEMM needs no on-chip transposes or casts.
- Device per core:
    Phase A (GEMM, up-front): pre[dir] = Wc[dir] @ xT + bc. 16 chunks of
      512 rows; 2 accumulating matmuls per (dir, gate-block); psum
      evicted to bf16 `pre` with the bias added (alternating ACT/DVE).
    Phase B (scan): layout-C recurrence, two offset chains (f ahead, b
      half-step behind) sharing the engines. Gates [128 feat, 4x16] in
      PSUM: identity-matmul accumulates pre, 4 small matmuls add Whh@h.
      All gate nonlinearities via ONE tanh per chain-step (i,f,o rows
      pre-scaled by 0.5 so sigmoid(x) = 0.5*tanh(x/2)+0.5 is an affine
      fixup fused into the DVE affine_mul_reduce ops).
    Phase C (LayerNorm, interleaved into the scan): hs transposed back
      to row space via xbar DMA on the Sync queue only; mean/var and
      the normalization run entirely on DVE (rsqrt via bitcast-shift
      Newton, no ACT table switches); output DMA on Sync.
"""

import sys

for _p in ("/opt/trn_rl_repo", "/opt/pypackages"):
    if _p not in sys.path:
        sys.path.insert(0, _p)

from contextlib import ExitStack

import ml_dtypes
import numpy as np

import concourse.bacc as bacc
import concourse.mybir as mybir
import concourse.tile as tile
from concourse.bass_utils import run_bass_kernel_spmd

BF = mybir.dt.bfloat16
F32 = mybir.dt.float32
I32 = mybir.dt.int32
AF = mybir.ActivationFunctionType
ALU = mybir.AluOpType

H = 128
DD = 256
G4 = 512
N_CORES = 8
EPS = 1e-5
MAGIC = 0x5F3759DF

_BUILD_CACHE = {}


def build_nc(L=512, BL=16, n_cores=N_CORES):
    key = (L, BL, n_cores)
    if key in _BUILD_CACHE:
        return _BUILD_CACHE[key]
    R = BL * L  # rows per core (row index = t*BL + b)
    CH = 512 if R >= 512 else R  # GEMM chunk rows
    NCH = R // CH
    nc = bacc.Bacc("TRN2", target_bir_lowering=False, debug=False, num_devices=n_cores)

    xt_d = [
        nc.dram_tensor(f"xt{k}", [128, R], BF, kind="ExternalInput").ap()
        for k in range(2)
    ]
    wc_d = {
        d: [
            nc.dram_tensor(f"wc_{d}{k}", [128, G4], BF, kind="ExternalInput").ap()
            for k in range(2)
        ]
        for d in "fb"
    }
    whh_d = {
        d: nc.dram_tensor(f"whh_{d}", [128, G4], BF, kind="ExternalInput").ap()
        for d in "fb"
    }
    bc_d = {
        d: nc.dram_tensor(f"bc_{d}", [128, 4], F32, kind="ExternalInput").ap()
        for d in "fb"
    }
    ident_d = nc.dram_tensor("ident", [128, 128], BF, kind="ExternalInput").ap()
    y = nc.dram_tensor("y", [BL, L, 2 * H], F32, kind="ExternalOutput").ap()

    y3 = y.rearrange("b l f -> l b f")

    with tile.TileContext(nc) as tc:
        with ExitStack() as ctx:
            # --- persistent pools (whole kernel) ---
            wpool = ctx.enter_context(tc.tile_pool(name="w", bufs=1))
            prepool = ctx.enter_context(tc.tile_pool(name="pre", bufs=1))
            hspool = ctx.enter_context(tc.tile_pool(name="hs", bufs=1))
            stpool = ctx.enter_context(tc.tile_pool(name="state", bufs=1))

            # load weights to SBUF
            wc_sb = {}
            whh_sb = {}
            bc_sb = {}
            for d in "fb":
                wc_sb[d] = []
                for k in range(2):
                    t_ = wpool.tile([128, G4], BF, tag=f"wc{d}{k}")
                    nc.sync.dma_start(t_[:], wc_d[d][k])
                    wc_sb[d].append(t_)
                t_ = wpool.tile([128, G4], BF, tag=f"whh{d}")
                nc.sync.dma_start(t_[:], whh_d[d])
                whh_sb[d] = t_
                t_ = wpool.tile([128, 4], F32, tag=f"bc{d}")
                nc.sync.dma_start(t_[:], bc_d[d])
                bc_sb[d] = t_
            ident = wpool.tile([128, 128], BF, tag="ident")
            nc.sync.dma_start(ident[:], ident_d)

            # big SBUF tensors
            pre = {d: prepool.tile([128, L * 64], BF, tag=f"pre{d}", name=f"pre{d}") for d in "fb"}
            hs = {d: hspool.tile([128, R], BF, tag=f"hs{d}", name=f"hs{d}") for d in "fb"}
            c_sb = {d: stpool.tile([128, BL], F32, tag=f"c{d}", name=f"c{d}") for d in "fb"}

            # 4D views of pre: [p, t, m, b]
            pre_v = {
                d: pre[d][:].rearrange("p (t m b) -> p t m b", m=4, b=BL) for d in "fb"
            }

            # ------- Phase A: GEMM (no transposes; fine-interleaved) -------
            # First chunks (0, NCH-1, 1) run up-front so both scan chains
            # have a full chunk of `pre` plus margin; the rest interleave
            # into the scan at ~1 big matmul per scan pair (PE slack).
            xtp = ctx.enter_context(tc.tile_pool(name="xt", bufs=6))
            gpsum = ctx.enter_context(tc.tile_pool(name="gpsum", bufs=2, space="PSUM"))
            gemm_psum = {}

            def gemm_dma(ch):
                xk = [xtp.tile([128, CH], BF, name=f"x{k}") for k in range(2)]
                for k in range(2):
                    nc.sync.dma_start(xk[k][:], xt_d[k][:, ch * CH : (ch + 1) * CH])
                return xk

            def gemm_mm(xk, ch, d, m, k):
                if k == 0:
                    gemm_psum[(ch, d, m)] = gpsum.tile([128, CH], F32, name="gps")
                ps = gemm_psum[(ch, d, m)]
                nc.tensor.matmul(
                    ps[:],
                    wc_sb[d][k][:, m * 128 : (m + 1) * 128],
                    xk[k][:],
                    start=(k == 0),
                    stop=(k == 1),
                )

            def gemm_evict(ch, d, m):
                ps = gemm_psum.pop((ch, d, m))
                t0 = ch * (CH // BL)
                nt = CH // BL
                dst = pre_v[d][:, t0 : t0 + nt, m, :]
                bias_ap = bc_sb[d][:, m : m + 1]
                if (m + (d == "b")) % 2 == 0:
                    nc.scalar.activation(dst, ps[:], AF.Identity, bias=bias_ap)
                else:
                    nc.vector.tensor_scalar_add(dst, ps[:], bias_ap)

            def gemm_chunk_full(ch):
                xk = gemm_dma(ch)
                for d in "fb":
                    for m in range(4):
                        gemm_mm(xk, ch, d, m, 0)
                        gemm_mm(xk, ch, d, m, 1)
                        gemm_evict(ch, d, m)

            lo, hi = 0, NCH - 1
            pp_order = []
            while lo <= hi:
                pp_order.append(lo)
                if hi != lo:
                    pp_order.append(hi)
                lo += 1
                hi -= 1
            upfront, rest = pp_order[:3], pp_order[3:]
            for ch in upfront:
                gemm_chunk_full(ch)

            gemm_work = []  # ("dma", ch) | ("mm", ch, d, m, k) | ("ev", ch, d, m)
            for ch in rest:
                gemm_work.append(("dma", ch))
                for d in "fb":
                    for m in range(4):
                        gemm_work.append(("mm", ch, d, m, 0))
                        gemm_work.append(("mm", ch, d, m, 1))
                        gemm_work.append(("ev", ch, d, m))
            gemm_xk = {}

            def gemm_pump():
                mm_done = 0
                while gemm_work and mm_done < 1:
                    item = gemm_work.pop(0)
                    if item[0] == "dma":
                        gemm_xk[item[1]] = gemm_dma(item[1])
                    elif item[0] == "mm":
                        _, ch, d, m, k = item
                        gemm_mm(gemm_xk[ch], ch, d, m, k)
                        mm_done += 1
                    else:
                        _, ch, d, m = item
                        gemm_evict(ch, d, m)

            # ------- Phase B: scan (dir b runs a half-step behind dir f) ----
            spsum = {
                d: ctx.enter_context(tc.tile_pool(name=f"ps{d}", bufs=3, space="PSUM"))
                for d in "fb"
            }
            tap = ctx.enter_context(tc.tile_pool(name="ta", bufs=4))
            uvp = ctx.enter_context(tc.tile_pool(name="uv", bufs=4))
            thp = ctx.enter_context(tc.tile_pool(name="th", bufs=4))
            junkp = ctx.enter_context(tc.tile_pool(name="junk", bufs=8))

            def emit_mms(d, s):
                t = s if d == "f" else L - 1 - s
                t_prev = (s - 1) if d == "f" else (L - s)
                ps = spsum[d].tile([128, 64], F32, name=f"ps{d}")
                nc.tensor.matmul(
                    ps[:],
                    ident[:],
                    pre[d][:, t * 64 : (t + 1) * 64],
                    start=True,
                    stop=True,
                )
                if s > 0:
                    h_prev = hs[d][:, t_prev * BL : (t_prev + 1) * BL]
                    for m in range(4):
                        nc.tensor.matmul(
                            ps[:, m * BL : (m + 1) * BL],
                            whh_sb[d][:, m * 128 : (m + 1) * 128],
                            h_prev,
                            start=False,
                            stop=True,
                            skip_group_check=True,
                        )
                return ps, t

            def emit_tanh_all(d, ps):
                ta = tap.tile([128, 64], F32, tag=f"ta{d}", name=f"ta{d}")
                nc.scalar.activation(ta[:], ps[:], AF.Tanh)
                return ta

            def emit_uva(d, s, ta):
                u = uvp.tile([128, BL], F32, tag=f"u{d}", name=f"u{d}")
                ju = junkp.tile([128, 1], F32, name="ju")
                nc.vector.affine_mul_reduce(
                    u[:], ju[:], ta[:, 0:BL], ta[:, 48:64], 0.5, 0.5
                )
                if s == 0:
                    nc.vector.tensor_copy(c_sb[d][:], u[:])
                else:
                    v = uvp.tile([128, BL], F32, tag=f"v{d}", name=f"v{d}")
                    jv = junkp.tile([128, 1], F32, name="jv")
                    nc.vector.affine_mul_reduce(
                        v[:], jv[:], ta[:, BL : 2 * BL], c_sb[d][:], 0.5, 0.5
                    )
                    nc.vector.tensor_add(c_sb[d][:], u[:], v[:])

            def emit_tc(d):
                th = thp.tile([128, BL], F32, tag=f"th{d}", name=f"th{d}")
                nc.scalar.activation(th[:], c_sb[d][:], AF.Tanh)
                return th

            def emit_h(d, t, ta, th):
                jh = junkp.tile([128, 1], F32, name="jh")
                nc.vector.affine_mul_reduce(
                    hs[d][:, t * BL : (t + 1) * BL],
                    jh[:],
                    ta[:, 2 * BL : 3 * BL],
                    th[:],
                    0.5,
                    0.5,
                )

            # ------- Phase C: LayerNorm (DVE-only compute, staged) -------
            xrp = ctx.enter_context(tc.tile_pool(name="xr", bufs=8))
            scp = ctx.enter_context(tc.tile_pool(name="scr", bufs=2))
            statp = ctx.enter_context(tc.tile_pool(name="stat", bufs=8))
            intp = ctx.enter_context(tc.tile_pool(name="int", bufs=4))
            outp = ctx.enter_context(tc.tile_pool(name="out", bufs=4))
            NF = 2 * H

            def ln_xbar(st):
                xr = xrp.tile([128, NF], BF, name="xr")
                cc = st["cc"]
                for di, d in enumerate("fb"):
                    nc.sync.dma_start_transpose(
                        xr[:, di * H : (di + 1) * H],
                        hs[d][:, cc * 128 : (cc + 1) * 128],
                    )
                st["xr"] = xr

            def ln_stats(st):
                xr = st["xr"]
                scr = scp.tile([128, NF], F32, name="scr")
                s2 = statp.tile([128, 1], F32, tag="s2", name="s2")
                nc.vector.affine_mul_reduce(scr[:], s2[:], xr[:], xr[:], 1.0, 0.0)
                s1 = statp.tile([128, 1], F32, tag="s1", name="s1")
                nc.vector.tensor_reduce(
                    s1[:], xr[:], axis=mybir.AxisListType.X, op=ALU.add
                )
                st["s1"], st["s2"] = s1, s2

            def ln_var(st):
                # tiny [128,1] stat ops run on the idle GpSimd engine
                s1, s2 = st["s1"], st["s2"]
                mu = statp.tile([128, 1], F32, tag="mu", name="mu")
                nc.gpsimd.tensor_scalar_mul(mu[:], s1[:], 1.0 / NF)
                e2 = statp.tile([128, 1], F32, tag="e2", name="e2")
                nc.gpsimd.tensor_scalar(e2[:], s2[:], 1.0 / NF, EPS, ALU.mult, ALU.add)
                mu2 = statp.tile([128, 1], F32, tag="mu2", name="mu2")
                nc.gpsimd.tensor_mul(mu2[:], mu[:], mu[:])
                w = statp.tile([128, 1], F32, tag="w", name="w")
                nc.gpsimd.tensor_sub(w[:], e2[:], mu2[:])
                st["mu"], st["w"] = mu, w

            def ln_seed(st):
                # int bit-trick seed: DVE only (gpsimd rejects int alu ops)
                w = st["w"]
                yv = intp.tile([128, 1], I32, name="yi")
                nc.vector.tensor_scalar(
                    yv[:], w[:].bitcast(I32), 1, None, ALU.arith_shift_right
                )
                nc.vector.tensor_scalar(yv[:], yv[:], -1, None, ALU.bitwise_xor)
                nc.vector.tensor_scalar(yv[:], yv[:], MAGIC + 1, None, ALU.add)
                st["y"] = yv

            def ln_newton(st):
                w, yv = st["w"], st["y"]
                yf = yv[:].bitcast(F32)
                a = statp.tile([128, 1], F32, tag="na", name="na")
                nc.gpsimd.tensor_mul(a[:], w[:], yf)
                nc.gpsimd.tensor_mul(a[:], a[:], yf)
                nc.gpsimd.tensor_scalar(a[:], a[:], -0.5, 1.5, ALU.mult, ALU.add)
                r = statp.tile([128, 1], F32, tag="r", name="r")
                nc.gpsimd.tensor_mul(r[:], yf, a[:])
                # second Newton iteration for safety margin
                a2 = statp.tile([128, 1], F32, tag="na2", name="na2")
                nc.gpsimd.tensor_mul(a2[:], w[:], r[:])
                nc.gpsimd.tensor_mul(a2[:], a2[:], r[:])
                nc.gpsimd.tensor_scalar(a2[:], a2[:], -0.5, 1.5, ALU.mult, ALU.add)
                nc.gpsimd.tensor_mul(r[:], r[:], a2[:])
                st["r"] = r

            def ln_affine(st):
                xr, mu, r, cc = st["xr"], st["mu"], st["r"], st["cc"]
                nmr = statp.tile([128, 1], F32, tag="nmr", name="nmr")
                nc.vector.scalar_tensor_tensor(
                    nmr[:], mu[:], -1.0, r[:], ALU.mult, ALU.mult
                )
                ot = outp.tile([128, NF], F32, name="ot")
                nc.vector.tensor_scalar(ot[:], xr[:], r[:], nmr[:], ALU.mult, ALU.add)
                nl = 128 // BL
                nc.sync.dma_start(y3[cc * nl : (cc + 1) * nl], ot[:])

            LN_STAGES = [ln_xbar, ln_stats, ln_var, ln_seed, ln_newton, ln_affine]

            # readiness: chunk cc needs hs_f t<8(cc+1) (s >= 8cc+7) and
            # hs_b t>=8cc (hs_b last write for the block is EMITTED one
            # iteration later due to the skew -> s >= L-8cc)
            ln_at = {}
            ln_post = []
            for cc in range(R // 128):
                s_ready = max(8 * cc + 7, L - 8 * cc)
                if s_ready <= L - 1 - len(LN_STAGES):
                    ln_at.setdefault(s_ready, []).append(cc)
                else:
                    ln_post.append(cc)

            ln_q = []  # pending (state dict with stage index)

            def ln_pump(s, max_stages):
                # at most one stage per chunk per call; stats waits >=2
                # steps after the xbar transposes were issued
                done = 0
                for st in list(ln_q):
                    if done >= max_stages:
                        break
                    if st["stage"] == 1 and s < st["s_xbar"] + 2:
                        continue
                    if st["stage"] == 0:
                        st["s_xbar"] = s
                    LN_STAGES[st["stage"]](st)
                    st["stage"] += 1
                    done += 1
                    if st["stage"] >= len(LN_STAGES):
                        ln_q.remove(st)

            pend = None  # (ta_b, t_b) awaiting tail emission
            for s in range(L):
                ps_f, t_f = emit_mms("f", s)
                ta_f = emit_tanh_all("f", ps_f)
                if pend is not None:
                    ta_bp, t_bp = pend
                    th_b = emit_tc("b")
                    emit_h("b", t_bp, ta_bp, th_b)
                ps_b, t_b = emit_mms("b", s)
                emit_uva("f", s, ta_f)
                ta_b = emit_tanh_all("b", ps_b)
                emit_uva("b", s, ta_b)
                th_f = emit_tc("f")
                emit_h("f", t_f, ta_f, th_f)
                pend = (ta_b, t_b)
                gemm_pump()
                for cc in ln_at.get(s, []):
                    ln_q.append({"cc": cc, "stage": 0})
                ln_pump(s, 2)
            ta_bp, t_bp = pend
            th_b = emit_tc("b")
            emit_h("b", t_bp, ta_bp, th_b)
            ln_pump(10**9, 10**9)
            while ln_q:
                ln_pump(10**9, 10**9)
            for cc in ln_post:
                st = {"cc": cc, "stage": 0}
                for fn in LN_STAGES:
                    fn(st)

    nc.compile()
    _BUILD_CACHE[key] = nc
    return nc


def _prep_weights(W_proj, b_proj, Wih, Whh, b):
    """Host-side: fold projection, permute gates to (i,f,o,g), pre-scale
    i/f/o rows by 0.5 (sigmoid-via-tanh trick), build lhsT layouts."""
    perm = np.r_[0:256, 384:512, 256:384]
    scale = np.concatenate([np.full(384, 0.5), np.ones(128)]).astype(np.float64)
    Wc = (Wih.astype(np.float64) @ W_proj.astype(np.float64))[perm] * scale[:, None]
    bc = (Wih.astype(np.float64) @ b_proj.astype(np.float64) + b.astype(np.float64))[
        perm
    ] * scale
    Whh_p = Whh[perm].astype(np.float64) * scale[:, None]
    bf16 = ml_dtypes.bfloat16
    WcT = np.ascontiguousarray(Wc.T.astype(np.float32).astype(bf16))  # [D, 4H]
    WhhT = np.ascontiguousarray(Whh_p.T.astype(np.float32).astype(bf16))  # [H, 4H]
    bc128 = np.ascontiguousarray(bc.astype(np.float32).reshape(4, 128).T)  # [128, 4]
    return WcT, WhhT, bc128


def _prep_x_shard(x_shard):
    """[BL, L, D] f32 -> two [128, L*BL] bf16 feature-major halves."""
    BLs, L, D = x_shard.shape
    bf16 = ml_dtypes.bfloat16
    xT = np.ascontiguousarray(
        x_shard.transpose(2, 1, 0).reshape(D, L * BLs).astype(bf16)
    )
    return np.ascontiguousarray(xT[0:128]), np.ascontiguousarray(xT[128:256])


def kernel(x, W_proj, b_proj, Wih_f, Whh_f, b_f, Wih_b, Whh_b, b_b, gamma, beta):
    x = np.asarray(x, dtype=np.float32)
    B, L, D = x.shape
    BL = B // N_CORES
    nc = build_nc(L=L, BL=BL)

    bf16 = ml_dtypes.bfloat16
    in_common = {"ident": np.eye(128, dtype=np.float32).astype(bf16)}
    for d, Wih, Whh, b in (("f", Wih_f, Whh_f, b_f), ("b", Wih_b, Whh_b, b_b)):
        WcT, WhhT, bc128 = _prep_weights(
            np.asarray(W_proj), np.asarray(b_proj), np.asarray(Wih), np.asarray(Whh),
            np.asarray(b),
        )
        in_common[f"wc_{d}0"] = np.ascontiguousarray(WcT[0:128])
        in_common[f"wc_{d}1"] = np.ascontiguousarray(WcT[128:256])
        in_common[f"whh_{d}"] = WhhT
        in_common[f"bc_{d}"] = bc128

    in_maps = []
    for i in range(N_CORES):
        xt0, xt1 = _prep_x_shard(x[i * BL : (i + 1) * BL])
        in_maps.append({**in_common, "xt0": xt0, "xt1": xt1})
    res = run_bass_kernel_spmd(nc, in_maps, list(range(N_CORES)))
    out = np.concatenate([res.results[i]["y"] for i in range(N_CORES)], axis=0)

    gamma = np.asarray(gamma, dtype=np.float32)
    beta = np.asarray(beta, dtype=np.float32)
    if not (np.all(gamma == 1.0) and np.all(beta == 0.0)):
        out = out * gamma + beta
    return out.astype(np.float32)


if __name__ == "__main__":
    d = np.load("/root/problem/ref.npz")
    inp = {k: d[k] for k in d.files if k != "exp"}
    got = kernel(**inp)
    exp = d["exp"]
    rel = np.linalg.norm(got - exp) / np.linalg.norm(exp)
    print("rel fro:", rel, "maxabs:", np.abs(got - exp).max())


# revision 11
# speedup vs baseline: 1.2008x; 1.2008x over previous
"""BiLSTM encoder Trainium2 kernel (8-core SPMD, batch-data-parallel).

Strategy (v2)
-------------
- Shard batch B=128 across 8 cores (16 per core); replicate weights.
- Host folds the input projection into the gate projection and
  pre-transposes x to feature-major bf16:
      pre = Wc @ x^T + bc,   Wc = (Wih @ W_proj) (permuted/scaled)
  so the device GEMM needs no on-chip transposes or casts.
- Device per core:
    Phase A (GEMM, up-front): pre[dir] = Wc[dir] @ xT + bc. 16 chunks of
      512 rows; 2 accumulating matmuls per (dir, gate-block); psum
      evicted to bf16 `pre` with the bias added (alternating ACT/DVE).
    Phase B (scan): layout-C recurrence, two offset chains (f ahead, b
      half-step behind) sharing the engines. Gates [128 feat, 4x16] in
      PSUM: identity-matmul accumulates pre, 4 small matmuls add Whh@h.
      All gate nonlinearities via ONE tanh per chain-step (i,f,o rows
      pre-scaled by 0.5 so sigmoid(x) = 0.5*tanh(x/2)+0.5 is an affine
      fixup fused into the DVE affine_mul_reduce ops).
    Phase C (LayerNorm, interleaved into the scan): hs transposed back
      to row space via xbar DMA on the Sync queue only; mean/var and
      the normalization run entirely on DVE (rsqrt via bitcast-shift
      Newton, no ACT table switches); output DMA on Sync.
"""

import sys

for _p in ("/opt/trn_rl_repo", "/opt/pypackages"):
    if _p not in sys.path:
        sys.path.insert(0, _p)

from contextlib import ExitStack

import ml_dtypes
import numpy as np

import concourse.bacc as bacc
import concourse.mybir as mybir
import concourse.tile as tile
from concourse.bass_utils import run_bass_kernel_spmd

BF = mybir.dt.bfloat16
F32 = mybir.dt.float32
I32 = mybir.dt.int32
AF = mybir.ActivationFunctionType
ALU = mybir.AluOpType

H = 128
DD = 256
G4 = 512
N_CORES = 8
EPS = 1e-5
MAGIC = 0x5F3759DF

_BUILD_CACHE = {}


def build_nc(L=512, BL=16, n_cores=N_CORES):
    key = (L, BL, n_cores)
    if key in _BUILD_CACHE:
        return _BUILD_CACHE[key]
    R = BL * L  # rows per core (row index = t*BL + b)
    CH = 512 if R >= 512 else R  # GEMM chunk rows
    NCH = R // CH
    nc = bacc.Bacc("TRN2", target_bir_lowering=False, debug=False, num_devices=n_cores)

    xt_d = [
        nc.dram_tensor(f"xt{k}", [128, R], BF, kind="ExternalInput").ap()
        for k in range(2)
    ]
    wc_d = {
        d: [
            nc.dram_tensor(f"wc_{d}{k}", [128, G4], BF, kind="ExternalInput").ap()
            for k in range(2)
        ]
        for d in "fb"
    }
    whh_d = {
        d: nc.dram_tensor(f"whh_{d}", [128, G4], BF, kind="ExternalInput").ap()
        for d in "fb"
    }
    bc_d = {
        d: nc.dram_tensor(f"bc_{d}", [128, 4], F32, kind="ExternalInput").ap()
        for d in "fb"
    }
    ident_d = nc.dram_tensor("ident", [128, 128], BF, kind="ExternalInput").ap()
    y = nc.dram_tensor("y", [BL, L, 2 * H], F32, kind="ExternalOutput").ap()

    y3 = y.rearrange("b l f -> l b f")

    with tile.TileContext(nc) as tc:
        with ExitStack() as ctx:
            # --- persistent pools (whole kernel) ---
            wpool = ctx.enter_context(tc.tile_pool(name="w", bufs=1))
            prepool = ctx.enter_context(tc.tile_pool(name="pre", bufs=1))
            hspool = ctx.enter_context(tc.tile_pool(name="hs", bufs=1))
            stpool = ctx.enter_context(tc.tile_pool(name="state", bufs=1))

            # load weights to SBUF
            wc_sb = {}
            whh_sb = {}
            bc_sb = {}
            for d in "fb":
                wc_sb[d] = []
                for k in range(2):
                    t_ = wpool.tile([128, G4], BF, tag=f"wc{d}{k}")
                    nc.sync.dma_start(t_[:], wc_d[d][k])
                    wc_sb[d].append(t_)
                t_ = wpool.tile([128, G4], BF, tag=f"whh{d}")
                nc.sync.dma_start(t_[:], whh_d[d])
                whh_sb[d] = t_
                t_ = wpool.tile([128, 4], F32, tag=f"bc{d}")
                nc.sync.dma_start(t_[:], bc_d[d])
                bc_sb[d] = t_
            ident = wpool.tile([128, 128], BF, tag="ident")
            nc.sync.dma_start(ident[:], ident_d)

            # big SBUF tensors
            pre = {d: prepool.tile([128, L * 64], BF, tag=f"pre{d}", name=f"pre{d}") for d in "fb"}
            hs = {d: hspool.tile([128, R], BF, tag=f"hs{d}", name=f"hs{d}") for d in "fb"}
            c_sb = {d: stpool.tile([128, BL], F32, tag=f"c{d}", name=f"c{d}") for d in "fb"}

            # 4D views of pre: [p, t, m, b]
            pre_v = {
                d: pre[d][:].rearrange("p (t m b) -> p t m b", m=4, b=BL) for d in "fb"
            }

            # ------- Phase A: GEMM (up-front, no transposes) -------
            # Up-front keeps the big matmuls back-to-back (PE stays warm);
            # interleaving them into the scan measured slower on HW (cold
            # matmuls inflate the scan's PE FIFO).
            xtp = ctx.enter_context(tc.tile_pool(name="xt", bufs=6))
            gpsum = ctx.enter_context(tc.tile_pool(name="gpsum", bufs=2, space="PSUM"))

            for ch in range(NCH):
                xk = [xtp.tile([128, CH], BF, name=f"x{k}") for k in range(2)]
                for k in range(2):
                    nc.sync.dma_start(xk[k][:], xt_d[k][:, ch * CH : (ch + 1) * CH])
                t0 = ch * (CH // BL)
                nt = CH // BL
                for d in "fb":
                    for m in range(4):
                        ps = gpsum.tile([128, CH], F32, name="gps")
                        nc.tensor.matmul(
                            ps[:],
                            wc_sb[d][0][:, m * 128 : (m + 1) * 128],
                            xk[0][:],
                            start=True,
                            stop=False,
                        )
                        nc.tensor.matmul(
                            ps[:],
                            wc_sb[d][1][:, m * 128 : (m + 1) * 128],
                            xk[1][:],
                            start=False,
                            stop=True,
                        )
                        dst = pre_v[d][:, t0 : t0 + nt, m, :]
                        bias_ap = bc_sb[d][:, m : m + 1]
                        if (m + (d == "b")) % 2 == 0:
                            nc.scalar.activation(dst, ps[:], AF.Identity, bias=bias_ap)
                        else:
                            nc.vector.tensor_scalar_add(dst, ps[:], bias_ap)

            # ------- Phase B: scan (dir b runs a half-step behind dir f) ----
            spsum = {
                d: ctx.enter_context(tc.tile_pool(name=f"ps{d}", bufs=3, space="PSUM"))
                for d in "fb"
            }
            tap = ctx.enter_context(tc.tile_pool(name="ta", bufs=4))
            uvp = ctx.enter_context(tc.tile_pool(name="uv", bufs=4))
            thp = ctx.enter_context(tc.tile_pool(name="th", bufs=4))
            junkp = ctx.enter_context(tc.tile_pool(name="junk", bufs=8))

            def emit_mms(d, s):
                t = s if d == "f" else L - 1 - s
                t_prev = (s - 1) if d == "f" else (L - s)
                ps = spsum[d].tile([128, 64], F32, name=f"ps{d}")
                nc.tensor.matmul(
                    ps[:],
                    ident[:],
                    pre[d][:, t * 64 : (t + 1) * 64],
                    start=True,
                    stop=True,
                )
                if s > 0:
                    h_prev = hs[d][:, t_prev * BL : (t_prev + 1) * BL]
                    for m in range(4):
                        nc.tensor.matmul(
                            ps[:, m * BL : (m + 1) * BL],
                            whh_sb[d][:, m * 128 : (m + 1) * 128],
                            h_prev,
                            start=False,
                            stop=True,
                            skip_group_check=True,
                        )
                return ps, t

            def emit_tanh_all(d, ps):
                ta = tap.tile([128, 64], F32, tag=f"ta{d}", name=f"ta{d}")
                nc.scalar.activation(ta[:], ps[:], AF.Tanh)
                return ta

            def emit_uva(d, s, ta):
                u = uvp.tile([128, BL], F32, tag=f"u{d}", name=f"u{d}")
                ju = junkp.tile([128, 1], F32, name="ju")
                nc.vector.affine_mul_reduce(
                    u[:], ju[:], ta[:, 0:BL], ta[:, 48:64], 0.5, 0.5
                )
                if s == 0:
                    nc.vector.tensor_copy(c_sb[d][:], u[:])
                else:
                    v = uvp.tile([128, BL], F32, tag=f"v{d}", name=f"v{d}")
                    jv = junkp.tile([128, 1], F32, name="jv")
                    nc.vector.affine_mul_reduce(
                        v[:], jv[:], ta[:, BL : 2 * BL], c_sb[d][:], 0.5, 0.5
                    )
                    nc.vector.tensor_add(c_sb[d][:], u[:], v[:])

            def emit_tc(d):
                th = thp.tile([128, BL], F32, tag=f"th{d}", name=f"th{d}")
                nc.scalar.activation(th[:], c_sb[d][:], AF.Tanh)
                return th

            def emit_h(d, t, ta, th):
                jh = junkp.tile([128, 1], F32, name="jh")
                nc.vector.affine_mul_reduce(
                    hs[d][:, t * BL : (t + 1) * BL],
                    jh[:],
                    ta[:, 2 * BL : 3 * BL],
                    th[:],
                    0.5,
                    0.5,
                )

            # ------- Phase C: LayerNorm (DVE-only compute, staged) -------
            xrp = ctx.enter_context(tc.tile_pool(name="xr", bufs=8))
            scp = ctx.enter_context(tc.tile_pool(name="scr", bufs=2))
            statp = ctx.enter_context(tc.tile_pool(name="stat", bufs=8))
            intp = ctx.enter_context(tc.tile_pool(name="int", bufs=4))
            outp = ctx.enter_context(tc.tile_pool(name="out", bufs=4))
            NF = 2 * H

            def ln_xbar(st):
                xr = xrp.tile([128, NF], BF, name="xr")
                cc = st["cc"]
                for di, d in enumerate("fb"):
                    nc.sync.dma_start_transpose(
                        xr[:, di * H : (di + 1) * H],
                        hs[d][:, cc * 128 : (cc + 1) * 128],
                    )
                st["xr"] = xr

            def ln_stats(st):
                xr = st["xr"]
                scr = scp.tile([128, NF], F32, name="scr")
                s2 = statp.tile([128, 1], F32, tag="s2", name="s2")
                nc.vector.affine_mul_reduce(scr[:], s2[:], xr[:], xr[:], 1.0, 0.0)
                s1 = statp.tile([128, 1], F32, tag="s1", name="s1")
                nc.vector.tensor_reduce(
                    s1[:], xr[:], axis=mybir.AxisListType.X, op=ALU.add
                )
                st["s1"], st["s2"] = s1, s2

            def ln_var(st):
                # all-DVE: cross-engine hops here stall the DVE FIFO (HW-measured)
                s1, s2 = st["s1"], st["s2"]
                mu = statp.tile([128, 1], F32, tag="mu", name="mu")
                nc.vector.tensor_scalar_mul(mu[:], s1[:], 1.0 / NF)
                e2 = statp.tile([128, 1], F32, tag="e2", name="e2")
                nc.vector.tensor_scalar(e2[:], s2[:], 1.0 / NF, EPS, ALU.mult, ALU.add)
                mu2 = statp.tile([128, 1], F32, tag="mu2", name="mu2")
                nc.vector.tensor_mul(mu2[:], mu[:], mu[:])
                w = statp.tile([128, 1], F32, tag="w", name="w")
                nc.vector.scalar_tensor_tensor(
                    w[:], mu2[:], -1.0, e2[:], ALU.mult, ALU.add
                )
                st["mu"], st["w"] = mu, w

            def ln_seed(st):
                # int bit-trick seed: DVE only (gpsimd rejects int alu ops)
                w = st["w"]
                yv = intp.tile([128, 1], I32, name="yi")
                nc.vector.tensor_scalar(
                    yv[:], w[:].bitcast(I32), 1, None, ALU.arith_shift_right
                )
                nc.vector.tensor_scalar(yv[:], yv[:], -1, None, ALU.bitwise_xor)
                nc.vector.tensor_scalar(yv[:], yv[:], MAGIC + 1, None, ALU.add)
                st["y"] = yv

            def ln_newton(st):
                # one Newton iteration: quake seed err <=3.4% -> <=0.2% after
                # one pass, far inside the accuracy budget
                w, yv = st["w"], st["y"]
                yf = yv[:].bitcast(F32)
                a = statp.tile([128, 1], F32, tag="na", name="na")
                nc.vector.tensor_mul(a[:], w[:], yf)
                nc.vector.tensor_mul(a[:], a[:], yf)
                nc.vector.tensor_scalar(a[:], a[:], -0.5, 1.5, ALU.mult, ALU.add)
                r = statp.tile([128, 1], F32, tag="r", name="r")
                nc.vector.tensor_mul(r[:], yf, a[:])
                st["r"] = r

            def ln_affine(st):
                xr, mu, r, cc = st["xr"], st["mu"], st["r"], st["cc"]
                nmr = statp.tile([128, 1], F32, tag="nmr", name="nmr")
                nc.vector.scalar_tensor_tensor(
                    nmr[:], mu[:], -1.0, r[:], ALU.mult, ALU.mult
                )
                ot = outp.tile([128, NF], F32, name="ot")
                # ACT has slack in the LN region; DVE is the binding engine
                nc.scalar.activation(ot[:], xr[:], AF.Identity, bias=nmr[:], scale=r[:])
                nl = 128 // BL
                nc.sync.dma_start(y3[cc * nl : (cc + 1) * nl], ot[:])

            LN_STAGES = [ln_xbar, ln_stats, ln_var, ln_seed, ln_newton, ln_affine]

            # readiness: chunk cc needs hs_f t<8(cc+1) (s >= 8cc+7) and
            # hs_b t>=8cc (hs_b last write for the block is EMITTED one
            # iteration later due to the skew -> s >= L-8cc)
            ln_at = {}
            ln_post = []
            for cc in range(R // 128):
                s_ready = max(8 * cc + 7, L - 8 * cc)
                if s_ready <= L - 1 - len(LN_STAGES):
                    ln_at.setdefault(s_ready, []).append(cc)
                else:
                    ln_post.append(cc)

            ln_q = []  # pending (state dict with stage index)

            def ln_pump(s, max_stages):
                # at most one stage per chunk per call; stats waits >=2
                # steps after the xbar transposes were issued
                done = 0
                for st in list(ln_q):
                    if done >= max_stages:
                        break
                    if st["stage"] == 1 and s < st["s_xbar"] + 2:
                        continue
                    if st["stage"] == 0:
                        st["s_xbar"] = s
                    LN_STAGES[st["stage"]](st)
                    st["stage"] += 1
                    done += 1
                    if st["stage"] >= len(LN_STAGES):
                        ln_q.remove(st)

            pend = None  # (ta_b, t_b) awaiting tail emission
            for s in range(L):
                ps_f, t_f = emit_mms("f", s)
                ta_f = emit_tanh_all("f", ps_f)
                if pend is not None:
                    ta_bp, t_bp = pend
                    th_b = emit_tc("b")
                    emit_h("b", t_bp, ta_bp, th_b)
                ps_b, t_b = emit_mms("b", s)
                emit_uva("f", s, ta_f)
                ta_b = emit_tanh_all("b", ps_b)
                emit_uva("b", s, ta_b)
                th_f = emit_tc("f")
                emit_h("f", t_f, ta_f, th_f)
                pend = (ta_b, t_b)
                for cc in ln_at.get(s, []):
                    ln_q.append({"cc": cc, "stage": 0})
                ln_pump(s, 2)
            ta_bp, t_bp = pend
            th_b = emit_tc("b")
            emit_h("b", t_bp, ta_bp, th_b)
            ln_pump(10**9, 10**9)
            while ln_q:
                ln_pump(10**9, 10**9)
            for cc in ln_post:
                st = {"cc": cc, "stage": 0}
                for fn in LN_STAGES:
                    fn(st)

    nc.compile()
    _BUILD_CACHE[key] = nc
    return nc


def _prep_weights(W_proj, b_proj, Wih, Whh, b):
    """Host-side: fold projection, permute gates to (i,f,o,g), pre-scale
    i/f/o rows by 0.5 (sigmoid-via-tanh trick), build lhsT layouts."""
    perm = np.r_[0:256, 384:512, 256:384]
    scale = np.concatenate([np.full(384, 0.5), np.ones(128)]).astype(np.float64)
    Wc = (Wih.astype(np.float64) @ W_proj.astype(np.float64))[perm] * scale[:, None]
    bc = (Wih.astype(np.float64) @ b_proj.astype(np.float64) + b.astype(np.float64))[
        perm
    ] * scale
    Whh_p = Whh[perm].astype(np.float64) * scale[:, None]
    bf16 = ml_dtypes.bfloat16
    WcT = np.ascontiguousarray(Wc.T.astype(np.float32).astype(bf16))  # [D, 4H]
    WhhT = np.ascontiguousarray(Whh_p.T.astype(np.float32).astype(bf16))  # [H, 4H]
    bc128 = np.ascontiguousarray(bc.astype(np.float32).reshape(4, 128).T)  # [128, 4]
    return WcT, WhhT, bc128


def _prep_x_shard(x_shard):
    """[BL, L, D] f32 -> two [128, L*BL] bf16 feature-major halves."""
    BLs, L, D = x_shard.shape
    bf16 = ml_dtypes.bfloat16
    xT = np.ascontiguousarray(
        x_shard.transpose(2, 1, 0).reshape(D, L * BLs).astype(bf16)
    )
    return np.ascontiguousarray(xT[0:128]), np.ascontiguousarray(xT[128:256])


def kernel(x, W_proj, b_proj, Wih_f, Whh_f, b_f, Wih_b, Whh_b, b_b, gamma, beta):
    x = np.asarray(x, dtype=np.float32)
    B, L, D = x.shape
    BL = B // N_CORES
    nc = build_nc(L=L, BL=BL)

    bf16 = ml_dtypes.bfloat16
    in_common = {"ident": np.eye(128, dtype=np.float32).astype(bf16)}
    for d, Wih, Whh, b in (("f", Wih_f, Whh_f, b_f), ("b", Wih_b, Whh_b, b_b)):
        WcT, WhhT, bc128 = _prep_weights(
            np.asarray(W_proj), np.asarray(b_proj), np.asarray(Wih), np.asarray(Whh),
            np.asarray(b),
        )
        in_common[f"wc_{d}0"] = np.ascontiguousarray(WcT[0:128])
        in_common[f"wc_{d}1"] = np.ascontiguousarray(WcT[128:256])
        in_common[f"whh_{d}"] = WhhT
        in_common[f"bc_{d}"] = bc128

    in_maps = []
    for i in range(N_CORES):
        xt0, xt1 = _prep_x_shard(x[i * BL : (i + 1) * BL])
        in_maps.append({**in_common, "xt0": xt0, "xt1": xt1})
    res = run_bass_kernel_spmd(nc, in_maps, list(range(N_CORES)))
    out = np.concatenate([res.results[i]["y"] for i in range(N_CORES)], axis=0)

    gamma = np.asarray(gamma, dtype=np.float32)
    beta = np.asarray(beta, dtype=np.float32)
    if not (np.all(gamma == 1.0) and np.all(beta == 0.0)):
        out = out * gamma + beta
    return out.astype(np.float32)


if __name__ == "__main__":
    d = np.load("/root/problem/ref.npz")
    inp = {k: d[k] for k in d.files if k != "exp"}
    got = kernel(**inp)
    exp = d["exp"]
    rel = np.linalg.norm(got - exp) / np.linalg.norm(exp)
    print("rel fro:", rel, "maxabs:", np.abs(got - exp).max())


# revision 17
# speedup vs baseline: 1.2329x; 1.0267x over previous
"""BiLSTM encoder Trainium2 kernel (8-core SPMD, batch-data-parallel).

Strategy (v2)
-------------
- Shard batch B=128 across 8 cores (16 per core); replicate weights.
- Host folds the input projection into the gate projection and
  pre-transposes x to feature-major bf16:
      pre = Wc @ x^T + bc,   Wc = (Wih @ W_proj) (permuted/scaled)
  so the device GEMM needs no on-chip transposes or casts.
- Device per core:
    Phase A (GEMM, up-front): pre[dir] = Wc[dir] @ xT + bc. 16 chunks of
      512 rows; 2 accumulating matmuls per (dir, gate-block); psum
      evicted to bf16 `pre` with the bias added (alternating ACT/DVE).
    Phase B (scan): layout-C recurrence, two offset chains (f ahead, b
      half-step behind) sharing the engines. Gates [128 feat, 4x16] in
      PSUM: identity-matmul accumulates pre, 4 small matmuls add Whh@h.
      All gate nonlinearities via ONE tanh per chain-step (i,f,o rows
      pre-scaled by 0.5 so sigmoid(x) = 0.5*tanh(x/2)+0.5 is an affine
      fixup fused into the DVE affine_mul_reduce ops).
    Phase C (LayerNorm, interleaved into the scan): hs transposed back
      to row space via xbar DMA on the Sync queue only; mean/var and
      the normalization run entirely on DVE (rsqrt via bitcast-shift
      Newton, no ACT table switches); output DMA on Sync.
"""

import sys

for _p in ("/opt/trn_rl_repo", "/opt/pypackages"):
    if _p not in sys.path:
        sys.path.insert(0, _p)

from contextlib import ExitStack

import ml_dtypes
import numpy as np

import concourse.bacc as bacc
import concourse.mybir as mybir
import concourse.tile as tile
from concourse.bass_utils import run_bass_kernel_spmd

BF = mybir.dt.bfloat16
F32 = mybir.dt.float32
I32 = mybir.dt.int32
AF = mybir.ActivationFunctionType
ALU = mybir.AluOpType

H = 128
DD = 256
G4 = 512
N_CORES = 8
EPS = 1e-5
MAGIC = 0x5F3759DF

_BUILD_CACHE = {}


def build_nc(L=512, BL=16, n_cores=N_CORES):
    key = (L, BL, n_cores)
    if key in _BUILD_CACHE:
        return _BUILD_CACHE[key]
    R = BL * L  # rows per core (row index = t*BL + b)
    CH = 512 if R >= 512 else R  # GEMM chunk rows
    NCH = R // CH
    nc = bacc.Bacc("TRN2", target_bir_lowering=False, debug=False, num_devices=n_cores)

    xt_d = [
        nc.dram_tensor(f"xt{k}", [128, R], BF, kind="ExternalInput").ap()
        for k in range(2)
    ]
    wc_d = {
        d: [
            nc.dram_tensor(f"wc_{d}{k}", [128, G4], BF, kind="ExternalInput").ap()
            for k in range(2)
        ]
        for d in "fb"
    }
    whh_d = {
        d: nc.dram_tensor(f"whh_{d}", [128, G4], BF, kind="ExternalInput").ap()
        for d in "fb"
    }
    bc_d = {
        d: nc.dram_tensor(f"bc_{d}", [128, 4], F32, kind="ExternalInput").ap()
        for d in "fb"
    }
    ident_d = nc.dram_tensor("ident", [128, 128], BF, kind="ExternalInput").ap()
    y = nc.dram_tensor("y", [BL, L, 2 * H], F32, kind="ExternalOutput").ap()

    y3 = y.rearrange("b l f -> l b f")

    with tile.TileContext(nc) as tc:
        with ExitStack() as ctx:
            # --- persistent pools (whole kernel) ---
            wpool = ctx.enter_context(tc.tile_pool(name="w", bufs=1))
            prepool = ctx.enter_context(tc.tile_pool(name="pre", bufs=1))
            hspool = ctx.enter_context(tc.tile_pool(name="hs", bufs=1))
            stpool = ctx.enter_context(tc.tile_pool(name="state", bufs=1))

            # load weights to SBUF
            wc_sb = {}
            whh_sb = {}
            bc_sb = {}
            for d in "fb":
                wc_sb[d] = []
                for k in range(2):
                    t_ = wpool.tile([128, G4], BF, tag=f"wc{d}{k}")
                    nc.sync.dma_start(t_[:], wc_d[d][k])
                    wc_sb[d].append(t_)
                t_ = wpool.tile([128, G4], BF, tag=f"whh{d}")
                nc.sync.dma_start(t_[:], whh_d[d])
                whh_sb[d] = t_
                t_ = wpool.tile([128, 4], F32, tag=f"bc{d}")
                nc.sync.dma_start(t_[:], bc_d[d])
                bc_sb[d] = t_
            ident = wpool.tile([128, 128], BF, tag="ident")
            nc.sync.dma_start(ident[:], ident_d)

            # big SBUF tensors
            pre = {d: prepool.tile([128, L * 64], BF, tag=f"pre{d}", name=f"pre{d}") for d in "fb"}
            hs = {d: hspool.tile([128, R], BF, tag=f"hs{d}", name=f"hs{d}") for d in "fb"}
            c_sb = {d: stpool.tile([128, BL], F32, tag=f"c{d}", name=f"c{d}") for d in "fb"}

            # 4D views of pre: [p, t, m, b]
            pre_v = {
                d: pre[d][:].rearrange("p (t m b) -> p t m b", m=4, b=BL) for d in "fb"
            }

            # ------- Phase A: GEMM (up-front, no transposes) -------
            # Up-front keeps the big matmuls back-to-back (PE stays warm);
            # interleaving them into the scan measured slower on HW (cold
            # matmuls inflate the scan's PE FIFO).
            xtp = ctx.enter_context(tc.tile_pool(name="xt", bufs=6))
            gpsum = ctx.enter_context(tc.tile_pool(name="gpsum", bufs=2, space="PSUM"))

            for ch in range(NCH):
                xk = [xtp.tile([128, CH], BF, name=f"x{k}") for k in range(2)]
                for k in range(2):
                    nc.sync.dma_start(xk[k][:], xt_d[k][:, ch * CH : (ch + 1) * CH])
                t0 = ch * (CH // BL)
                nt = CH // BL
                for d in "fb":
                    for m in range(4):
                        ps = gpsum.tile([128, CH], F32, name="gps")
                        nc.tensor.matmul(
                            ps[:],
                            wc_sb[d][0][:, m * 128 : (m + 1) * 128],
                            xk[0][:],
                            start=True,
                            stop=False,
                        )
                        nc.tensor.matmul(
                            ps[:],
                            wc_sb[d][1][:, m * 128 : (m + 1) * 128],
                            xk[1][:],
                            start=False,
                            stop=True,
                        )
                        dst = pre_v[d][:, t0 : t0 + nt, m, :]
                        bias_ap = bc_sb[d][:, m : m + 1]
                        if (m + (d == "b")) % 2 == 0:
                            nc.scalar.activation(dst, ps[:], AF.Identity, bias=bias_ap)
                        else:
                            nc.vector.tensor_scalar_add(dst, ps[:], bias_ap)

            # ------- Phase B: scan (dir b runs a half-step behind dir f) ----
            spsum = {
                d: ctx.enter_context(tc.tile_pool(name=f"ps{d}", bufs=3, space="PSUM"))
                for d in "fb"
            }
            tap = ctx.enter_context(tc.tile_pool(name="ta", bufs=4))
            uvp = ctx.enter_context(tc.tile_pool(name="uv", bufs=4))
            thp = ctx.enter_context(tc.tile_pool(name="th", bufs=4))

            def emit_mms(d, s):
                t = s if d == "f" else L - 1 - s
                t_prev = (s - 1) if d == "f" else (L - s)
                ps = spsum[d].tile([128, 64], F32, name=f"ps{d}")
                nc.tensor.matmul(
                    ps[:],
                    ident[:],
                    pre[d][:, t * 64 : (t + 1) * 64],
                    start=True,
                    stop=True,
                )
                if s > 0:
                    h_prev = hs[d][:, t_prev * BL : (t_prev + 1) * BL]
                    for m in range(4):
                        nc.tensor.matmul(
                            ps[:, m * BL : (m + 1) * BL],
                            whh_sb[d][:, m * 128 : (m + 1) * 128],
                            h_prev,
                            start=False,
                            stop=True,
                            skip_group_check=True,
                        )
                return ps, t

            def emit_tanh_all(d, ps):
                ta = tap.tile([128, 64], F32, tag=f"ta{d}", name=f"ta{d}")
                nc.scalar.activation(ta[:], ps[:], AF.Tanh)
                return ta

            # Scaled cell state: the tile `c_sb` holds C = 2c. With
            # sig(x) = (tanh(x/2)+1)/2 and STT's (in0 op s) op in1 shape:
            #   u' = (ta_i + 1) * tanh(g)      = 2 sig(i) tanh(g) = 2u
            #   v' = (ta_f + 1) * C            = 2 sig(f) C
            #   C  = 0.5 v' + u'               = sig(f) C + 2u = 2c  ✓
            # tanh(c) = tanh(0.5 * C) via ACT's free input scale; h is
            # stored as 2h (Whh is host-scaled by 0.5; LayerNorm is
            # scale-invariant so the output is unchanged).
            def emit_uva(d, s, ta):
                if s == 0:
                    nc.vector.scalar_tensor_tensor(
                        c_sb[d][:], ta[:, 0:BL], 1.0, ta[:, 48:64], ALU.add, ALU.mult
                    )
                else:
                    u = uvp.tile([128, BL], F32, tag=f"u{d}", name=f"u{d}")
                    nc.vector.scalar_tensor_tensor(
                        u[:], ta[:, 0:BL], 1.0, ta[:, 48:64], ALU.add, ALU.mult
                    )
                    v = uvp.tile([128, BL], F32, tag=f"v{d}", name=f"v{d}")
                    nc.vector.scalar_tensor_tensor(
                        v[:], ta[:, BL : 2 * BL], 1.0, c_sb[d][:], ALU.add, ALU.mult
                    )
                    nc.vector.scalar_tensor_tensor(
                        c_sb[d][:], v[:], 0.5, u[:], ALU.mult, ALU.add
                    )

            def emit_tc(d):
                th = thp.tile([128, BL], F32, tag=f"th{d}", name=f"th{d}")
                nc.scalar.activation(th[:], c_sb[d][:], AF.Tanh, scale=0.5)
                return th

            def emit_h(d, t, ta, th):
                nc.vector.scalar_tensor_tensor(
                    hs[d][:, t * BL : (t + 1) * BL],
                    ta[:, 2 * BL : 3 * BL],
                    1.0,
                    th[:],
                    ALU.add,
                    ALU.mult,
                )

            # ------- Phase C: LayerNorm (DVE-only compute, staged) -------
            xrp = ctx.enter_context(tc.tile_pool(name="xr", bufs=8))
            scp = ctx.enter_context(tc.tile_pool(name="scr", bufs=2))
            statp = ctx.enter_context(tc.tile_pool(name="stat", bufs=8))
            intp = ctx.enter_context(tc.tile_pool(name="int", bufs=4))
            outp = ctx.enter_context(tc.tile_pool(name="out", bufs=4))
            NF = 2 * H
            eps = statp.tile([128, 1], F32, tag="eps", bufs=1, name="eps")
            nc.vector.memset(eps[:], EPS)

            def ln_xbar(st):
                xr = xrp.tile([128, NF], BF, name="xr")
                cc = st["cc"]
                for di, d in enumerate("fb"):
                    nc.sync.dma_start_transpose(
                        xr[:, di * H : (di + 1) * H],
                        hs[d][:, cc * 128 : (cc + 1) * 128],
                    )
                st["xr"] = xr

            def ln_stats(st):
                xr = st["xr"]
                scr = scp.tile([128, NF], F32, name="scr")
                s2 = statp.tile([128, 1], F32, tag="s2", name="s2")
                nc.vector.affine_mul_reduce(scr[:], s2[:], xr[:], xr[:], 1.0, 0.0)
                s1 = statp.tile([128, 1], F32, tag="s1", name="s1")
                nc.vector.tensor_reduce(
                    s1[:], xr[:], axis=mybir.AxisListType.X, op=ALU.add
                )
                # mu/e2 on ACT (emitted a full stage before their DVE
                # consumers, so the DVE FIFO never waits on the ACT queue)
                mu = statp.tile([128, 1], F32, tag="mu", name="mu")
                nc.scalar.activation(mu[:], s1[:], AF.Identity, scale=1.0 / NF)
                e2 = statp.tile([128, 1], F32, tag="e2", name="e2")
                nc.scalar.activation(e2[:], s2[:], AF.Identity, bias=eps[:], scale=1.0 / NF)
                st["mu"], st["e2"] = mu, e2

            def ln_var(st):
                mu, e2 = st["mu"], st["e2"]
                mu2 = statp.tile([128, 1], F32, tag="mu2", name="mu2")
                nc.vector.tensor_mul(mu2[:], mu[:], mu[:])
                w = statp.tile([128, 1], F32, tag="w", name="w")
                nc.vector.scalar_tensor_tensor(
                    w[:], mu2[:], -1.0, e2[:], ALU.mult, ALU.add
                )
                st["w"] = w

            def ln_seed(st):
                # int bit-trick seed: DVE only (gpsimd rejects int alu ops)
                w = st["w"]
                yv = intp.tile([128, 1], I32, name="yi")
                nc.vector.tensor_scalar(
                    yv[:], w[:].bitcast(I32), 1, None, ALU.arith_shift_right
                )
                nc.vector.tensor_scalar(yv[:], yv[:], -1, None, ALU.bitwise_xor)
                nc.vector.tensor_scalar(yv[:], yv[:], MAGIC + 1, None, ALU.add)
                st["y"] = yv

            def ln_newton(st):
                # one Newton iteration: quake seed err <=3.4% -> <=0.2% after
                # one pass, far inside the accuracy budget
                w, yv = st["w"], st["y"]
                yf = yv[:].bitcast(F32)
                a = statp.tile([128, 1], F32, tag="na", name="na")
                nc.vector.tensor_mul(a[:], w[:], yf)
                nc.vector.tensor_mul(a[:], a[:], yf)
                nc.vector.tensor_scalar(a[:], a[:], -0.5, 1.5, ALU.mult, ALU.add)
                r = statp.tile([128, 1], F32, tag="r", name="r")
                nc.vector.tensor_mul(r[:], yf, a[:])
                st["r"] = r

            def ln_affine(st):
                xr, mu, r, cc = st["xr"], st["mu"], st["r"], st["cc"]
                nmr = statp.tile([128, 1], F32, tag="nmr", name="nmr")
                nc.vector.scalar_tensor_tensor(
                    nmr[:], mu[:], -1.0, r[:], ALU.mult, ALU.mult
                )
                ot = outp.tile([128, NF], F32, name="ot")
                # ACT has slack in the LN region; DVE is the binding engine
                nc.scalar.activation(ot[:], xr[:], AF.Identity, bias=nmr[:], scale=r[:])
                nl = 128 // BL
                nc.sync.dma_start(y3[cc * nl : (cc + 1) * nl], ot[:])

            LN_STAGES = [ln_xbar, ln_stats, ln_var, ln_seed, ln_newton, ln_affine]

            # readiness: chunk cc needs hs_f t<8(cc+1) (s >= 8cc+7) and
            # hs_b t>=8cc (hs_b last write for the block is EMITTED one
            # iteration later due to the skew -> s >= L-8cc)
            ln_at = {}
            ln_post = []
            for cc in range(R // 128):
                s_ready = max(8 * cc + 7, L - 8 * cc)
                if s_ready <= L - 1 - len(LN_STAGES):
                    ln_at.setdefault(s_ready, []).append(cc)
                else:
                    ln_post.append(cc)

            ln_q = []  # pending (state dict with stage index)

            def ln_pump(s, max_stages):
                # at most one stage per chunk per call; stats waits >=2
                # steps after the xbar transposes were issued
                done = 0
                for st in list(ln_q):
                    if done >= max_stages:
                        break
                    if st["stage"] == 1 and s < st["s_xbar"] + 2:
                        continue
                    if st["stage"] == 0:
                        st["s_xbar"] = s
                    LN_STAGES[st["stage"]](st)
                    st["stage"] += 1
                    done += 1
                    if st["stage"] >= len(LN_STAGES):
                        ln_q.remove(st)

            pend = None  # (ta_b, t_b) awaiting tail emission
            for s in range(L):
                ps_f, t_f = emit_mms("f", s)
                ta_f = emit_tanh_all("f", ps_f)
                if pend is not None:
                    ta_bp, t_bp = pend
                    th_b = emit_tc("b")
                    emit_h("b", t_bp, ta_bp, th_b)
                ps_b, t_b = emit_mms("b", s)
                emit_uva("f", s, ta_f)
                ta_b = emit_tanh_all("b", ps_b)
                emit_uva("b", s, ta_b)
                th_f = emit_tc("f")
                emit_h("f", t_f, ta_f, th_f)
                pend = (ta_b, t_b)
                for cc in ln_at.get(s, []):
                    ln_q.append({"cc": cc, "stage": 0})
                ln_pump(s, 2)
            ta_bp, t_bp = pend
            th_b = emit_tc("b")
            emit_h("b", t_bp, ta_bp, th_b)
            ln_pump(10**9, 10**9)
            while ln_q:
                ln_pump(10**9, 10**9)
            for cc in ln_post:
                st = {"cc": cc, "stage": 0}
                for fn in LN_STAGES:
                    fn(st)

    nc.compile()
    _BUILD_CACHE[key] = nc
    return nc


def _prep_weights(W_proj, b_proj, Wih, Whh, b):
    """Host-side: fold projection, permute gates to (i,f,o,g), pre-scale
    i/f/o rows by 0.5 (sigmoid-via-tanh trick), build lhsT layouts."""
    perm = np.r_[0:256, 384:512, 256:384]
    scale = np.concatenate([np.full(384, 0.5), np.ones(128)]).astype(np.float64)
    Wc = (Wih.astype(np.float64) @ W_proj.astype(np.float64))[perm] * scale[:, None]
    bc = (Wih.astype(np.float64) @ b_proj.astype(np.float64) + b.astype(np.float64))[
        perm
    ] * scale
    # extra 0.5: the device stores h scaled by 2 (STT identity trick)
    Whh_p = Whh[perm].astype(np.float64) * scale[:, None] * 0.5
    bf16 = ml_dtypes.bfloat16
    WcT = np.ascontiguousarray(Wc.T.astype(np.float32).astype(bf16))  # [D, 4H]
    WhhT = np.ascontiguousarray(Whh_p.T.astype(np.float32).astype(bf16))  # [H, 4H]
    bc128 = np.ascontiguousarray(bc.astype(np.float32).reshape(4, 128).T)  # [128, 4]
    return WcT, WhhT, bc128


def _prep_x_shard(x_shard):
    """[BL, L, D] f32 -> two [128, L*BL] bf16 feature-major halves."""
    BLs, L, D = x_shard.shape
    bf16 = ml_dtypes.bfloat16
    xT = np.ascontiguousarray(
        x_shard.transpose(2, 1, 0).reshape(D, L * BLs).astype(bf16)
    )
    return np.ascontiguousarray(xT[0:128]), np.ascontiguousarray(xT[128:256])


def kernel(x, W_proj, b_proj, Wih_f, Whh_f, b_f, Wih_b, Whh_b, b_b, gamma, beta):
    x = np.asarray(x, dtype=np.float32)
    B, L, D = x.shape
    BL = B // N_CORES
    nc = build_nc(L=L, BL=BL)

    bf16 = ml_dtypes.bfloat16
    in_common = {"ident": np.eye(128, dtype=np.float32).astype(bf16)}
    for d, Wih, Whh, b in (("f", Wih_f, Whh_f, b_f), ("b", Wih_b, Whh_b, b_b)):
        WcT, WhhT, bc128 = _prep_weights(
            np.asarray(W_proj), np.asarray(b_proj), np.asarray(Wih), np.asarray(Whh),
            np.asarray(b),
        )
        in_common[f"wc_{d}0"] = np.ascontiguousarray(WcT[0:128])
        in_common[f"wc_{d}1"] = np.ascontiguousarray(WcT[128:256])
        in_common[f"whh_{d}"] = WhhT
        in_common[f"bc_{d}"] = bc128

    in_maps = []
    for i in range(N_CORES):
        xt0, xt1 = _prep_x_shard(x[i * BL : (i + 1) * BL])
        in_maps.append({**in_common, "xt0": xt0, "xt1": xt1})
    res = run_bass_kernel_spmd(nc, in_maps, list(range(N_CORES)))
    out = np.concatenate([res.results[i]["y"] for i in range(N_CORES)], axis=0)

    gamma = np.asarray(gamma, dtype=np.float32)
    beta = np.asarray(beta, dtype=np.float32)
    if not (np.all(gamma == 1.0) and np.all(beta == 0.0)):
        out = out * gamma + beta
    return out.astype(np.float32)


if __name__ == "__main__":
    d = np.load("/root/problem/ref.npz")
    inp = {k: d[k] for k in d.files if k != "exp"}
    got = kernel(**inp)
    exp = d["exp"]
    rel = np.linalg.norm(got - exp) / np.linalg.norm(exp)
    print("rel fro:", rel, "maxabs:", np.abs(got - exp).max())


# revision 21
# speedup vs baseline: 1.2991x; 1.0537x over previous
"""BiLSTM encoder Trainium2 kernel (8-core SPMD, batch-data-parallel).

Strategy (v2)
-------------
- Shard batch B=128 across 8 cores (16 per core); replicate weights.
- Host folds the input projection into the gate projection and
  pre-transposes x to feature-major bf16:
      pre = Wc @ x^T + bc,   Wc = (Wih @ W_proj) (permuted/scaled)
  so the device GEMM needs no on-chip transposes or casts.
- Device per core:
    Phase A (GEMM, up-front): pre[dir] = Wc[dir] @ xT + bc. 16 chunks of
      512 rows; 2 accumulating matmuls per (dir, gate-block); psum
      evicted to bf16 `pre` with the bias added (alternating ACT/DVE).
    Phase B (scan): layout-C recurrence, two offset chains (f ahead, b
      half-step behind) sharing the engines. Gates [128 feat, 4x16] in
      PSUM: identity-matmul accumulates pre, 4 small matmuls add Whh@h.
      All gate nonlinearities via ONE tanh per chain-step (i,f,o rows
      pre-scaled by 0.5 so sigmoid(x) = 0.5*tanh(x/2)+0.5 is an affine
      fixup fused into the DVE affine_mul_reduce ops).
    Phase C (LayerNorm, interleaved into the scan): hs transposed back
      to row space via xbar DMA on the Sync queue only; mean/var and
      the normalization run entirely on DVE (rsqrt via bitcast-shift
      Newton, no ACT table switches); output DMA on Sync.
"""

import sys

for _p in ("/opt/trn_rl_repo", "/opt/pypackages"):
    if _p not in sys.path:
        sys.path.insert(0, _p)

from contextlib import ExitStack

import ml_dtypes
import numpy as np

import concourse.bacc as bacc
import concourse.mybir as mybir
import concourse.tile as tile
from concourse.bass_utils import run_bass_kernel_spmd

BF = mybir.dt.bfloat16
F32 = mybir.dt.float32
I32 = mybir.dt.int32
AF = mybir.ActivationFunctionType
ALU = mybir.AluOpType

H = 128
DD = 256
G4 = 512
N_CORES = 8
EPS = 1e-5
MAGIC = 0x5F3759DF

_BUILD_CACHE = {}


def build_nc(L=512, BL=16, n_cores=N_CORES):
    key = (L, BL, n_cores)
    if key in _BUILD_CACHE:
        return _BUILD_CACHE[key]
    R = BL * L  # rows per core (row index = t*BL + b)
    CH = 512 if R >= 512 else R  # GEMM chunk rows
    NCH = R // CH
    nc = bacc.Bacc("TRN2", target_bir_lowering=False, debug=False, num_devices=n_cores)

    xt_d = [
        nc.dram_tensor(f"xt{k}", [128, R], BF, kind="ExternalInput").ap()
        for k in range(2)
    ]
    wc_d = {
        d: [
            nc.dram_tensor(f"wc_{d}{k}", [128, G4], BF, kind="ExternalInput").ap()
            for k in range(2)
        ]
        for d in "fb"
    }
    whh_d = {
        d: nc.dram_tensor(f"whh_{d}", [128, G4], BF, kind="ExternalInput").ap()
        for d in "fb"
    }
    bc_d = {
        d: nc.dram_tensor(f"bc_{d}", [128, 4], F32, kind="ExternalInput").ap()
        for d in "fb"
    }
    ident_d = nc.dram_tensor("ident", [128, 128], BF, kind="ExternalInput").ap()
    y = nc.dram_tensor("y", [BL, L, 2 * H], F32, kind="ExternalOutput").ap()

    y3 = y.rearrange("b l f -> l b f")

    with tile.TileContext(nc) as tc:
        with ExitStack() as ctx:
            # --- persistent pools (whole kernel) ---
            wpool = ctx.enter_context(tc.tile_pool(name="w", bufs=1))
            prepool = ctx.enter_context(tc.tile_pool(name="pre", bufs=1))
            hspool = ctx.enter_context(tc.tile_pool(name="hs", bufs=1))
            stpool = ctx.enter_context(tc.tile_pool(name="state", bufs=1))

            # load weights to SBUF
            wc_sb = {}
            whh_sb = {}
            bc_sb = {}
            for d in "fb":
                wc_sb[d] = []
                for k in range(2):
                    t_ = wpool.tile([128, G4], BF, tag=f"wc{d}{k}")
                    nc.sync.dma_start(t_[:], wc_d[d][k])
                    wc_sb[d].append(t_)
                t_ = wpool.tile([128, G4], BF, tag=f"whh{d}")
                nc.sync.dma_start(t_[:], whh_d[d])
                whh_sb[d] = t_
                t_ = wpool.tile([128, 4], F32, tag=f"bc{d}")
                nc.sync.dma_start(t_[:], bc_d[d])
                bc_sb[d] = t_
            ident = wpool.tile([128, 128], BF, tag="ident")
            nc.sync.dma_start(ident[:], ident_d)

            # big SBUF tensors
            pre = {d: prepool.tile([128, L * 64], BF, tag=f"pre{d}", name=f"pre{d}") for d in "fb"}
            hs = {d: hspool.tile([128, R], BF, tag=f"hs{d}", name=f"hs{d}") for d in "fb"}
            # per-dir ping-pong tanh buffers; cols 0:64 = tanh(gates) of
            # step s, cols 64:80 = cell state C(s-1) written by the
            # previous step (adjacent to the g columns so u' and v' fuse
            # into one 32-wide STT)
            tab = {
                d: [
                    stpool.tile([128, 80], F32, tag=f"ta{d}{k}", name=f"ta{d}{k}")
                    for k in range(2)
                ]
                for d in "fb"
            }

            # 4D views of pre: [p, t, m, b]
            pre_v = {
                d: pre[d][:].rearrange("p (t m b) -> p t m b", m=4, b=BL) for d in "fb"
            }

            # ------- Phase A: GEMM (up-front, no transposes) -------
            # Up-front keeps the big matmuls back-to-back (PE stays warm);
            # interleaving them into the scan measured slower on HW (cold
            # matmuls inflate the scan's PE FIFO).
            xtp = ctx.enter_context(tc.tile_pool(name="xt", bufs=6))
            gpsum = ctx.enter_context(tc.tile_pool(name="gpsum", bufs=2, space="PSUM"))

            for ch in range(NCH):
                xk = [xtp.tile([128, CH], BF, name=f"x{k}") for k in range(2)]
                for k in range(2):
                    nc.sync.dma_start(xk[k][:], xt_d[k][:, ch * CH : (ch + 1) * CH])
                t0 = ch * (CH // BL)
                nt = CH // BL
                for d in "fb":
                    for m in range(4):
                        ps = gpsum.tile([128, CH], F32, name="gps")
                        nc.tensor.matmul(
                            ps[:],
                            wc_sb[d][0][:, m * 128 : (m + 1) * 128],
                            xk[0][:],
                            start=True,
                            stop=False,
                        )
                        nc.tensor.matmul(
                            ps[:],
                            wc_sb[d][1][:, m * 128 : (m + 1) * 128],
                            xk[1][:],
                            start=False,
                            stop=True,
                        )
                        dst = pre_v[d][:, t0 : t0 + nt, m, :]
                        bias_ap = bc_sb[d][:, m : m + 1]
                        if (m + (d == "b")) % 2 == 0:
                            nc.scalar.activation(dst, ps[:], AF.Identity, bias=bias_ap)
                        else:
                            nc.vector.tensor_scalar_add(dst, ps[:], bias_ap)

            # ------- Phase B: scan (dir b runs a half-step behind dir f) ----
            spsum = {
                d: ctx.enter_context(tc.tile_pool(name=f"ps{d}", bufs=3, space="PSUM"))
                for d in "fb"
            }
            tap = ctx.enter_context(tc.tile_pool(name="ta", bufs=4))
            uvp = ctx.enter_context(tc.tile_pool(name="uv", bufs=4))
            thp = ctx.enter_context(tc.tile_pool(name="th", bufs=4))

            def emit_mms(d, s):
                t = s if d == "f" else L - 1 - s
                t_prev = (s - 1) if d == "f" else (L - s)
                ps = spsum[d].tile([128, 64], F32, name=f"ps{d}")
                nc.tensor.matmul(
                    ps[:],
                    ident[:],
                    pre[d][:, t * 64 : (t + 1) * 64],
                    start=True,
                    stop=True,
                )
                if s > 0:
                    h_prev = hs[d][:, t_prev * BL : (t_prev + 1) * BL]
                    for m in range(4):
                        nc.tensor.matmul(
                            ps[:, m * BL : (m + 1) * BL],
                            whh_sb[d][:, m * 128 : (m + 1) * 128],
                            h_prev,
                            start=False,
                            stop=True,
                            skip_group_check=True,
                        )
                return ps, t

            def emit_tanh_all(d, s, ps):
                ta = tab[d][s % 2]
                nc.scalar.activation(ta[:, 0:64], ps[:], AF.Tanh)
                return ta

            # Scaled cell state: the tile `c_sb` holds C = 2c. With
            # sig(x) = (tanh(x/2)+1)/2 and STT's (in0 op s) op in1 shape:
            #   u' = (ta_i + 1) * tanh(g)      = 2 sig(i) tanh(g) = 2u
            #   v' = (ta_f + 1) * C            = 2 sig(f) C
            #   C  = 0.5 v' + u'               = sig(f) C + 2u = 2c  ✓
            # tanh(c) = tanh(0.5 * C) via ACT's free input scale; h is
            # stored as 2h (Whh is host-scaled by 0.5; LayerNorm is
            # scale-invariant so the output is unchanged).
            def emit_uva(d, s, ta):
                # C(s) lands in the NEXT step's buffer at cols 64:80
                cdst = tab[d][(s + 1) % 2][:, 64:80]
                if s == 0:
                    nc.vector.scalar_tensor_tensor(
                        cdst, ta[:, 0:BL], 1.0, ta[:, 48:64], ALU.add, ALU.mult
                    )
                else:
                    # one 32-wide STT: [u'|v'] = ([i|f] + 1) * [g|C(s-1)]
                    uv = uvp.tile([128, 2 * BL], F32, tag=f"uv{d}", name=f"uv{d}")
                    nc.vector.scalar_tensor_tensor(
                        uv[:], ta[:, 0 : 2 * BL], 1.0, ta[:, 48:80], ALU.add, ALU.mult
                    )
                    nc.vector.scalar_tensor_tensor(
                        cdst, uv[:, BL : 2 * BL], 0.5, uv[:, 0:BL], ALU.mult, ALU.add
                    )

            def emit_tc(d, s):
                th = thp.tile([128, BL], F32, tag=f"th{d}", name=f"th{d}")
                nc.scalar.activation(
                    th[:], tab[d][(s + 1) % 2][:, 64:80], AF.Tanh, scale=0.5
                )
                return th

            def emit_h(d, t, ta, th):
                nc.vector.scalar_tensor_tensor(
                    hs[d][:, t * BL : (t + 1) * BL],
                    ta[:, 2 * BL : 3 * BL],
                    1.0,
                    th[:],
                    ALU.add,
                    ALU.mult,
                )

            # ------- Phase C: LayerNorm (DVE-only compute, staged) -------
            xrp = ctx.enter_context(tc.tile_pool(name="xr", bufs=8))
            scp = ctx.enter_context(tc.tile_pool(name="scr", bufs=2))
            statp = ctx.enter_context(tc.tile_pool(name="stat", bufs=8))
            intp = ctx.enter_context(tc.tile_pool(name="int", bufs=4))
            outp = ctx.enter_context(tc.tile_pool(name="out", bufs=4))
            NF = 2 * H
            eps = statp.tile([128, 1], F32, tag="eps", bufs=1, name="eps")
            nc.vector.memset(eps[:], EPS)

            def ln_xbar(st):
                xr = xrp.tile([128, NF], BF, name="xr")
                cc = st["cc"]
                for di, d in enumerate("fb"):
                    nc.sync.dma_start_transpose(
                        xr[:, di * H : (di + 1) * H],
                        hs[d][:, cc * 128 : (cc + 1) * 128],
                    )
                st["xr"] = xr

            def ln_stats(st):
                xr = st["xr"]
                scr = scp.tile([128, NF], F32, name="scr")
                s2 = statp.tile([128, 1], F32, tag="s2", name="s2")
                nc.vector.affine_mul_reduce(scr[:], s2[:], xr[:], xr[:], 1.0, 0.0)
                s1 = statp.tile([128, 1], F32, tag="s1", name="s1")
                nc.vector.tensor_reduce(
                    s1[:], xr[:], axis=mybir.AxisListType.X, op=ALU.add
                )
                # mu/e2 on ACT (emitted a full stage before their DVE
                # consumers, so the DVE FIFO never waits on the ACT queue)
                mu = statp.tile([128, 1], F32, tag="mu", name="mu")
                nc.scalar.activation(mu[:], s1[:], AF.Identity, scale=1.0 / NF)
                e2 = statp.tile([128, 1], F32, tag="e2", name="e2")
                nc.scalar.activation(e2[:], s2[:], AF.Identity, bias=eps[:], scale=1.0 / NF)
                st["mu"], st["e2"] = mu, e2

            def ln_var(st):
                mu, e2 = st["mu"], st["e2"]
                mu2 = statp.tile([128, 1], F32, tag="mu2", name="mu2")
                nc.vector.tensor_mul(mu2[:], mu[:], mu[:])
                w = statp.tile([128, 1], F32, tag="w", name="w")
                nc.vector.scalar_tensor_tensor(
                    w[:], mu2[:], -1.0, e2[:], ALU.mult, ALU.add
                )
                st["w"] = w

            def ln_seed(st):
                # int bit-trick seed: DVE only (gpsimd rejects int alu ops)
                w = st["w"]
                yv = intp.tile([128, 1], I32, name="yi")
                nc.vector.tensor_scalar(
                    yv[:], w[:].bitcast(I32), 1, None, ALU.arith_shift_right
                )
                nc.vector.tensor_scalar(yv[:], yv[:], -1, None, ALU.bitwise_xor)
                nc.vector.tensor_scalar(yv[:], yv[:], MAGIC + 1, None, ALU.add)
                st["y"] = yv

            def ln_newton(st):
                # one Newton iteration: quake seed err <=3.4% -> <=0.2% after
                # one pass, far inside the accuracy budget
                w, yv = st["w"], st["y"]
                yf = yv[:].bitcast(F32)
                a = statp.tile([128, 1], F32, tag="na", name="na")
                nc.vector.tensor_mul(a[:], w[:], yf)
                nc.vector.tensor_mul(a[:], a[:], yf)
                nc.vector.tensor_scalar(a[:], a[:], -0.5, 1.5, ALU.mult, ALU.add)
                r = statp.tile([128, 1], F32, tag="r", name="r")
                nc.vector.tensor_mul(r[:], yf, a[:])
                st["r"] = r

            def ln_affine(st):
                xr, mu, r, cc = st["xr"], st["mu"], st["r"], st["cc"]
                nmr = statp.tile([128, 1], F32, tag="nmr", name="nmr")
                nc.vector.scalar_tensor_tensor(
                    nmr[:], mu[:], -1.0, r[:], ALU.mult, ALU.mult
                )
                ot = outp.tile([128, NF], F32, name="ot")
                # ACT has slack in the LN region; DVE is the binding engine
                nc.scalar.activation(ot[:], xr[:], AF.Identity, bias=nmr[:], scale=r[:])
                nl = 128 // BL
                nc.sync.dma_start(y3[cc * nl : (cc + 1) * nl], ot[:])

            LN_STAGES = [ln_xbar, ln_stats, ln_var, ln_seed, ln_newton, ln_affine]

            # readiness: chunk cc needs hs_f t<8(cc+1) (s >= 8cc+7) and
            # hs_b t>=8cc (hs_b last write for the block is EMITTED one
            # iteration later due to the skew -> s >= L-8cc)
            ln_at = {}
            ln_post = []
            for cc in range(R // 128):
                s_ready = max(8 * cc + 7, L - 8 * cc)
                if s_ready <= L - 1 - len(LN_STAGES):
                    ln_at.setdefault(s_ready, []).append(cc)
                else:
                    ln_post.append(cc)

            ln_q = []  # pending (state dict with stage index)

            def ln_pump(s, max_stages):
                # at most one stage per chunk per call; stats waits >=2
                # steps after the xbar transposes were issued
                done = 0
                for st in list(ln_q):
                    if done >= max_stages:
                        break
                    if st["stage"] == 1 and s < st["s_xbar"] + 2:
                        continue
                    if st["stage"] == 0:
                        st["s_xbar"] = s
                    LN_STAGES[st["stage"]](st)
                    st["stage"] += 1
                    done += 1
                    if st["stage"] >= len(LN_STAGES):
                        ln_q.remove(st)

            pend = None  # (ta_b, t_b, s_b) awaiting tail emission
            for s in range(L):
                ps_f, t_f = emit_mms("f", s)
                ta_f = emit_tanh_all("f", s, ps_f)
                if pend is not None:
                    ta_bp, t_bp, s_bp = pend
                    th_b = emit_tc("b", s_bp)
                    emit_h("b", t_bp, ta_bp, th_b)
                ps_b, t_b = emit_mms("b", s)
                emit_uva("f", s, ta_f)
                ta_b = emit_tanh_all("b", s, ps_b)
                emit_uva("b", s, ta_b)
                th_f = emit_tc("f", s)
                emit_h("f", t_f, ta_f, th_f)
                pend = (ta_b, t_b, s)
                for cc in ln_at.get(s, []):
                    ln_q.append({"cc": cc, "stage": 0})
                ln_pump(s, 2)
            ta_bp, t_bp, s_bp = pend
            th_b = emit_tc("b", s_bp)
            emit_h("b", t_bp, ta_bp, th_b)
            ln_pump(10**9, 10**9)
            while ln_q:
                ln_pump(10**9, 10**9)
            for cc in ln_post:
                st = {"cc": cc, "stage": 0}
                for fn in LN_STAGES:
                    fn(st)

    nc.compile()
    _BUILD_CACHE[key] = nc
    return nc


def _prep_weights(W_proj, b_proj, Wih, Whh, b):
    """Host-side: fold projection, permute gates to (i,f,o,g), pre-scale
    i/f/o rows by 0.5 (sigmoid-via-tanh trick), build lhsT layouts."""
    perm = np.r_[0:256, 384:512, 256:384]
    scale = np.concatenate([np.full(384, 0.5), np.ones(128)]).astype(np.float64)
    Wc = (Wih.astype(np.float64) @ W_proj.astype(np.float64))[perm] * scale[:, None]
    bc = (Wih.astype(np.float64) @ b_proj.astype(np.float64) + b.astype(np.float64))[
        perm
    ] * scale
    # extra 0.5: the device stores h scaled by 2 (STT identity trick)
    Whh_p = Whh[perm].astype(np.float64) * scale[:, None] * 0.5
    bf16 = ml_dtypes.bfloat16
    WcT = np.ascontiguousarray(Wc.T.astype(np.float32).astype(bf16))  # [D, 4H]
    WhhT = np.ascontiguousarray(Whh_p.T.astype(np.float32).astype(bf16))  # [H, 4H]
    bc128 = np.ascontiguousarray(bc.astype(np.float32).reshape(4, 128).T)  # [128, 4]
    return WcT, WhhT, bc128


def _prep_x_shard(x_shard):
    """[BL, L, D] f32 -> two [128, L*BL] bf16 feature-major halves."""
    BLs, L, D = x_shard.shape
    bf16 = ml_dtypes.bfloat16
    xT = np.ascontiguousarray(
        x_shard.transpose(2, 1, 0).reshape(D, L * BLs).astype(bf16)
    )
    return np.ascontiguousarray(xT[0:128]), np.ascontiguousarray(xT[128:256])


def kernel(x, W_proj, b_proj, Wih_f, Whh_f, b_f, Wih_b, Whh_b, b_b, gamma, beta):
    x = np.asarray(x, dtype=np.float32)
    B, L, D = x.shape
    BL = B // N_CORES
    nc = build_nc(L=L, BL=BL)

    bf16 = ml_dtypes.bfloat16
    in_common = {"ident": np.eye(128, dtype=np.float32).astype(bf16)}
    for d, Wih, Whh, b in (("f", Wih_f, Whh_f, b_f), ("b", Wih_b, Whh_b, b_b)):
        WcT, WhhT, bc128 = _prep_weights(
            np.asarray(W_proj), np.asarray(b_proj), np.asarray(Wih), np.asarray(Whh),
            np.asarray(b),
        )
        in_common[f"wc_{d}0"] = np.ascontiguousarray(WcT[0:128])
        in_common[f"wc_{d}1"] = np.ascontiguousarray(WcT[128:256])
        in_common[f"whh_{d}"] = WhhT
        in_common[f"bc_{d}"] = bc128

    in_maps = []
    for i in range(N_CORES):
        xt0, xt1 = _prep_x_shard(x[i * BL : (i + 1) * BL])
        in_maps.append({**in_common, "xt0": xt0, "xt1": xt1})
    res = run_bass_kernel_spmd(nc, in_maps, list(range(N_CORES)))
    out = np.concatenate([res.results[i]["y"] for i in range(N_CORES)], axis=0)

    gamma = np.asarray(gamma, dtype=np.float32)
    beta = np.asarray(beta, dtype=np.float32)
    if not (np.all(gamma == 1.0) and np.all(beta == 0.0)):
        out = out * gamma + beta
    return out.astype(np.float32)


if __name__ == "__main__":
    d = np.load("/root/problem/ref.npz")
    inp = {k: d[k] for k in d.files if k != "exp"}
    got = kernel(**inp)
    exp = d["exp"]
    rel = np.linalg.norm(got - exp) / np.linalg.norm(exp)
    print("rel fro:", rel, "maxabs:", np.abs(got - exp).max())
